# revision 2
# baseline (speedup 1.0000x reference)
"""Low-rank layer y = (U^T V) @ x computed as y = U^T @ (V @ x).

Full problem: x [8192, 4096] f32, U/V [8, 8192] f32, y [8192, 4096] f32.
Sharding: batch (columns of x) split across 8 NeuronCores, 512 per core.

Design (v3e):
- Per core the 512 batch columns are processed as two independent
  256-col halves, so phase 1 (t = V @ x) of half B overlaps phase 2
  (y = U^T t) of half A on the in-order PE stream: the PE runs
  back-to-back instead of serializing load-paced phase 1 against
  copy-paced phase 2.
- Phase 1 runs the PE with float32r operands: the moving operand
  streams at full rate for >=256-col matmuls, so x needs NO
  f32->bf16 cast anywhere (and accuracy beats bf16). One long 64-chunk
  PSUM accumulation per half.
- Phase 2 makes t the stationary operand ([8,128] slices) and streams
  U [8,512] bf16 tiles as the moving operand, producing y TRANSPOSED
  ([batch, L] tiles) straight into per-bank PSUM slices; the host
  un-transposes. No block-diagonal zero-padding/replication needed.
- DMA: few, large transfers on the two HWDGE rings (SP + ACT),
  alternating: 4 x 4 MiB f32 loads, 4 x 2 MiB bf16 stores per exec.
  Fewer DMAs per exec measurably beats many small ones at sustained
  rates (DMA-completion bookkeeping saturates at ~16 DMAs/exec).
- PSUM -> SBUF drain copies are split DVE:ACT 5:3 so neither engine
  paces above the PE. U/V stay resident across pipelined executions
  (consts loaded once per NEFF).
"""

import numpy as np

L = 8192
RANK = 8
BATCH = 4096
NCORES = 8
BS = BATCH // NCORES   # 512 batch columns per core
P = 128                # SBUF partitions
NCHUNK = L // P        # 64 row-chunks of 128
NH = 2                 # batch halves per core
HB = BS // NH          # 256 batch cols per half
NG = 2                 # x load groups per half (4 MiB each)
GC = NCHUNK // NG      # 32 chunks per load group
NBB = BS // P          # 4 batch sub-blocks of 128
STQ = 8                # 1024-col quarters per store (2 MiB)

_NC = None


def _body(tc, nc, x, vt, u, y, mybir, rep=1):
    from contextlib import ExitStack

    f32 = mybir.dt.float32
    f32r = mybir.dt.float32r
    bf16 = mybir.dt.bfloat16

    with ExitStack() as ctx:
        const = ctx.enter_context(tc.tile_pool(name="const", bufs=1))
        xpool = ctx.enter_context(tc.tile_pool(name="xb", bufs=3))
        tpsum = ctx.enter_context(tc.tile_pool(name="tpsum", bufs=2, space="PSUM"))
        tsb = ctx.enter_context(tc.tile_pool(name="tsb", bufs=2))
        ypsum = ctx.enter_context(tc.tile_pool(name="ypsum", bufs=3, space="PSUM"))
        ystage = ctx.enter_context(tc.tile_pool(name="ystage", bufs=3))

        vt_sb = const.tile([P, NCHUNK * RANK], f32r)
        nc.sync.dma_start(vt_sb[:], vt[:])
        u_sb = const.tile([RANK, L], bf16)
        nc.scalar.dma_start(u_sb[:], u[:])

        # Dummy matmuls absorbing the const-tensor DMA waits so the PE
        # queue head never blocks on them mid-stream.
        warm1 = tpsum.tile([RANK, RANK], f32, tag="t", name="warm1")
        nc.tensor.matmul(warm1[:], vt_sb[:, 0:RANK], vt_sb[:, 0:RANK],
                         start=True, stop=True)
        warm2 = tpsum.tile([P, RANK], f32, tag="t", name="warm2")
        nc.tensor.matmul(warm2[:], u_sb[:, 0:P], u_sb[:, 0:RANK],
                         start=True, stop=True)

        nload = 0
        nstore = 0
        ncopy = 0
        for r in range(rep):
            # All loads up front: 4 x 4 MiB, alternating HWDGE rings.
            xg = [[None] * NG for _ in range(NH)]
            for half in range(NH):
                for g in range(NG):
                    xt = xpool.tile([P, GC * HB], f32r, tag="xt",
                                    name=f"xg{r}_{half}_{g}")
                    eng = nc.sync if nload % 2 == 0 else nc.scalar
                    eng.dma_start(
                        xt[:], x[:, (half * NCHUNK + g * GC) * HB:
                                   (half * NCHUNK + (g + 1) * GC) * HB])
                    xg[half][g] = xt
                    nload += 1

            for half in range(NH):
                # Phase 1: t_half [8, 256] accumulated over 64 chunks.
                t_ps = tpsum.tile([RANK, HB], f32, tag="t", name=f"t{r}_{half}")
                for c in range(NCHUNK):
                    g, lc = c // GC, c % GC
                    nc.tensor.matmul(
                        t_ps[:],
                        vt_sb[:, c * RANK:(c + 1) * RANK],
                        xg[half][g][:, lc * HB:(lc + 1) * HB],
                        start=(c == 0),
                        stop=(c == NCHUNK - 1),
                        skip_group_check=True,
                    )
                t_sb = tsb.tile([RANK, HB], bf16, tag="tc", name=f"ts{r}_{half}")
                nc.vector.tensor_copy(t_sb[:], t_ps[:])

                # Phase 2: per 128-col batch sub-block, t_b stationary,
                # U streamed 1024 L-cols at a time; out y^T [batch, L].
                stage = None
                for sq in range(16):          # 16 x 1024 L-cols per half
                    sub, q = sq // 8, sq % 8
                    b = half * 2 + sub
                    if sq % STQ == 0:
                        stage = ystage.tile([P, STQ * 1024], bf16, tag="ys",
                                            name=f"ys{r}_{half}_{sq}")
                    y_ps = ypsum.tile([P, 1024], f32, tag="yp",
                                      name=f"yp{r}_{b}_{q}")
                    for h in range(2):
                        nc.tensor.matmul(
                            y_ps[:, h * 512:(h + 1) * 512],
                            t_sb[:, sub * P:(sub + 1) * P],
                            u_sb[:, (q * 2 + h) * 512:(q * 2 + h + 1) * 512],
                            start=True,
                            stop=True,
                        )
                    cp = (nc.vector.tensor_copy if ncopy % 8 < 5
                          else nc.scalar.copy)
                    ncopy += 1
                    cp(stage[:, (sq % STQ) * 1024:(sq % STQ + 1) * 1024],
                       y_ps[:])
                    if sq % STQ == STQ - 1:
                        off = (half * 2) * L + (sq // STQ) * (STQ * 1024)
                        eng = nc.sync if nstore % 2 == 0 else nc.scalar
                        eng.dma_start(y[:, off:off + STQ * 1024], stage[:])
                        nstore += 1


def build_bass(rep=1):
    import concourse.mybir as mybir
    import concourse.tile as tile
    from concourse import bacc

    nc = bacc.Bacc("TRN2", target_bir_lowering=False, debug=False)
    x = nc.dram_tensor("x", [P, NCHUNK * BS], mybir.dt.float32r,
                       kind="ExternalInput").ap()
    vt = nc.dram_tensor("vt", [P, NCHUNK * RANK], mybir.dt.float32r,
                        kind="ExternalInput").ap()
    u = nc.dram_tensor("u", [RANK, L], mybir.dt.bfloat16,
                       kind="ExternalInput").ap()
    y = nc.dram_tensor("y", [P, NBB * L], mybir.dt.bfloat16,
                       kind="ExternalOutput").ap()

    with tile.TileContext(nc) as tc:
        _body(tc, nc, x, vt, u, y, mybir, rep=rep)
    nc.compile()
    return nc


def _get_nc():
    global _NC
    if _NC is None:
        _NC = build_bass()
    return _NC


def make_in_maps(inputs, U, V):
    import ml_dtypes

    x = np.asarray(inputs, dtype=np.float32)
    U = np.asarray(U, dtype=np.float32)
    V = np.asarray(V, dtype=np.float32)
    vt = np.ascontiguousarray(
        V.reshape(RANK, NCHUNK, P).transpose(2, 1, 0).reshape(P, NCHUNK * RANK))
    ub = U.astype(ml_dtypes.bfloat16)
    in_maps = []
    for c in range(NCORES):
        xs = x[:, c * BS:(c + 1) * BS]
        # [p, half, chunk, col256]
        xb = np.ascontiguousarray(
            xs.reshape(NCHUNK, P, NH, HB).transpose(1, 2, 0, 3).reshape(P, -1))
        in_maps.append({"x": xb, "vt": vt, "u": ub})
    return in_maps


def _unblock_y(yb):
    # yb [P, NBB*L] bf16: yb[j, b*L + l] = y[l, b*128 + j]
    return np.ascontiguousarray(
        np.asarray(yb).reshape(P, NBB, L).transpose(2, 1, 0).reshape(L, BS)
    ).astype(np.float32)


def kernel(inputs, U, V):
    from concourse import bass_utils

    nc = _get_nc()
    in_maps = make_in_maps(inputs, U, V)
    res = bass_utils.run_bass_kernel_spmd(nc, in_maps, core_ids=list(range(NCORES)))
    return np.concatenate(
        [_unblock_y(res.results[c]["y"]) for c in range(NCORES)], axis=1)
